# revision 8
# baseline (speedup 1.0000x reference)
"""Trainium2 Bass kernel for CollapsePreventionLoss.

reference:
    atoms = coordinates.reshape(B, N, 3)           # B=64, N=1024
    dist  = sqrt(pairwise_dist_sq + 1e-8)
    loss  = sum_{i<j} relu(2.9 - dist)^2 / B

Strategy (8 NeuronCores, data parallel over batch, 8 batches/core):
  dist_sq[i,j] = s_i + s_j - 2 a_i.a_j  == one K=5 matmul with augmented
  vectors  x_i = [-2a_i, s_i, 1],  y_j = [a_j, 1, s_j + eps].
  Only the upper-triangle block-rows are computed: row-block r (128 rows)
  covers columns [128r, 1024).  The 128x128 diagonal block gets
  +BIG * lower_triangular added via a second (bf16, K=128) matmul so the
  subsequent relu(2.9 - sqrt(...)) is exactly 0 there.
  Per pair of row-blocks: PSUM tile -> ACT sqrt -> (per batch) DVE
  min(d-2.9, 0) -> DVE tensor_tensor_reduce(t*t, sum) -> [128,1] partials.
  Host sums partials in fp64.
"""

import sys

for _p in ("/opt/trn_rl_repo",):
    if _p not in sys.path:
        sys.path.insert(0, _p)

import numpy as np

import concourse.bass as bass
import concourse.bacc as bacc
import concourse.tile as tile
from concourse import mybir
from concourse.bass_utils import run_bass_kernel_spmd

B = 64
N = 1024
NCORES = 8
BPC = B // NCORES  # batches per core

MIN_DISTANCE = 2.9
LOSS_WEIGHT = 1.0
EPS_GUARD = 1e-4  # keeps dist_sq positive despite PSUM accumulation rounding
# (host emulation of the PE fp32 accumulation over this dataset bottoms out
# at dist_sq ~ -7.3e-6; 1e-4 gives >10x margin against sqrt(<0) = NaN)
BIG = 512.0  # added to lower+diag of diagonal blocks; relu kills it

# dist_sq is computed as an exact-in-fp32 bf16 hi/lo product expansion:
#   a ~= ah + al (bf16 pair), each product bf16 x bf16 is exact in fp32.
#   rows 0-2:   s_i  (3-way bf16 split)  x  1
#   rows 3-14:  4 products per coordinate: (-2ah,ah) (-2ah,al) (-2al,ah) (-2al,al)
#   rows 15-17: 1  x  (s_j + eps) (3-way bf16 split)
K_AUG = 18
P = 128
NRB = N // P  # row blocks per batch

# ---------------------------------------------------------------------------
# chunk plan: pairs of row blocks share one PSUM tile, packed tightly;
# chunks split at 512-column (bank) boundaries of the tile.
PAIRS = [(0, 1), (2, 3), (4, 5), (6, 7)]


def _plan():
    plan = []
    for ra, rb in PAIRS:
        wa, wb = N - P * ra, N - P * rb
        tile_w = wa + wb
        rows = [(ra, 0, wa), (rb, wa, wb)]
        chunks = []  # (r, roff, cstart_tile, width, is_first)
        for r, roff, w in rows:
            bounds = [roff]
            nb = (roff // 512 + 1) * 512
            while nb < roff + w:
                bounds.append(nb)
                nb += 512
            bounds.append(roff + w)
            for i in range(len(bounds) - 1):
                chunks.append((r, roff, bounds[i], bounds[i + 1] - bounds[i], i == 0))
        plan.append((tile_w, rows, chunks))
    return plan


PLAN = _plan()
D_W = sum(tw for tw, _, _ in PLAN)  # 4608: packed width of d per batch
D_OFF = np.cumsum([0] + [tw for tw, _, _ in PLAN])[:-1]

_cache = {}


def _build():
    if "nc" in _cache:
        return _cache["nc"]
    f32 = mybir.dt.float32
    bf16 = mybir.dt.bfloat16

    nc = bacc.Bacc("TRN2", target_bir_lowering=False, debug=False,
                   num_devices=NCORES)
    lhs_d = nc.dram_tensor("lhs", [BPC, K_AUG, N], bf16, kind="ExternalInput").ap()
    rhs_d = nc.dram_tensor("rhs", [BPC, K_AUG, N], bf16, kind="ExternalInput").ap()
    stepw_d = nc.dram_tensor("stepw", [P, P], bf16, kind="ExternalInput").ap()
    ident_d = nc.dram_tensor("ident", [P, P], bf16, kind="ExternalInput").ap()
    stats_d = nc.dram_tensor("stats", [P, BPC], f32, kind="ExternalOutput").ap()

    with tile.TileContext(nc) as tc:
        with (
            tc.tile_pool(name="consts", bufs=1) as consts,
            tc.tile_pool(name="inp", bufs=4) as inp,
            tc.tile_pool(name="dpool", bufs=2) as dpool,
            tc.tile_pool(name="tpool", bufs=2) as tpool,
            tc.tile_pool(name="sqpool", bufs=2) as sqpool,
            tc.tile_pool(name="spool", bufs=1) as spool,
            tc.tile_pool(name="psum", bufs=2, space="PSUM") as psum,
        ):
            step_sb = consts.tile([P, P], bf16, tag="step")
            ident_sb = consts.tile([P, P], bf16, tag="ident")
            nc.sync.dma_start(out=step_sb, in_=stepw_d)
            nc.sync.dma_start(out=ident_sb, in_=ident_d)
            stats_sb = spool.tile([P, BPC], f32, tag="stats")

            for b in range(BPC):
                lhs_sb = inp.tile([K_AUG, N], bf16, tag="lhs_t")
                rhs_sb = inp.tile([K_AUG, N], bf16, tag="rhs_t")
                nc.sync.dma_start(out=lhs_sb, in_=lhs_d[b])
                nc.sync.dma_start(out=rhs_sb, in_=rhs_d[b])
                lhs_mm = lhs_sb
                rhs_mm = rhs_sb

                d_sb = dpool.tile([P, D_W], f32, tag="d")
                for g, (tile_w, rows, chunks) in enumerate(PLAN):
                    pt = psum.tile([P, tile_w], f32, tag="pt")
                    for r, roff, cs, w, first in chunks:
                        jg = P * r + (cs - roff)  # global col of chunk start
                        nc.tensor.matmul(
                            pt[:, cs:cs + w],
                            lhs_mm[:, P * r:P * (r + 1)],
                            rhs_mm[:, jg:jg + w],
                            start=True,
                            stop=not first,
                        )
                        if first:
                            # add BIG*lower_tri(incl diag) onto the 128-wide
                            # diagonal block at [roff, roff+128)
                            nc.tensor.matmul(
                                pt[:, roff:roff + P],
                                step_sb,
                                ident_sb,
                                start=False,
                                stop=True,
                            )
                    nc.scalar.activation(
                        out=d_sb[:, int(D_OFF[g]):int(D_OFF[g]) + tile_w],
                        in_=pt[:, :],
                        func=mybir.ActivationFunctionType.Sqrt,
                        bias=0.0,  # EPS_GUARD is already inside dist_sq
                        scale=1.0,
                    )

                # t = min(d - 2.9, 0)   (== -relu(2.9 - d));  t*t == viol^2
                t_sb = tpool.tile([P, D_W], bf16, tag="t")
                nc.vector.tensor_scalar(
                    out=t_sb,
                    in0=d_sb,
                    scalar1=float(MIN_DISTANCE),
                    scalar2=0.0,
                    op0=mybir.AluOpType.subtract,
                    op1=mybir.AluOpType.min,
                )
                # (d - 2.9) * t == t*t (both factors equal t when d < 2.9,
                # and t == 0 otherwise); accum_out sums it per partition.
                sq_sb = sqpool.tile([P, D_W], bf16, tag="sq")
                nc.vector.scalar_tensor_tensor(
                    out=sq_sb,
                    in0=d_sb,
                    scalar=float(MIN_DISTANCE),
                    in1=t_sb,
                    op0=mybir.AluOpType.subtract,
                    op1=mybir.AluOpType.mult,
                    accum_out=stats_sb[:, b:b + 1],
                )

            nc.sync.dma_start(out=stats_d, in_=stats_sb)

    nc.compile()
    _cache["nc"] = nc
    return nc


def _bf16_split(x, n):
    """Split fp64 array into n bf16 terms summing to ~x."""
    import ml_dtypes

    out = []
    rem = x.copy()
    for _ in range(n):
        h = rem.astype(ml_dtypes.bfloat16)
        out.append(h)
        rem = rem - h.astype(np.float64)
    return out


def _prep_inputs(coords):
    """Host-side: build augmented lhs/rhs per core (bf16 hi/lo expansion)."""
    import ml_dtypes

    bf = ml_dtypes.bfloat16
    atoms = coords.reshape(B, N, 3).astype(np.float64)
    at = atoms.transpose(0, 2, 1)  # [B, 3, N]
    ah = at.astype(bf)
    al = (at - ah.astype(np.float64)).astype(bf)
    a_eff = ah.astype(np.float64) + al.astype(np.float64)
    s_eff = (a_eff * a_eff).sum(axis=1)  # [B, N] exact squared norms of ã

    si = _bf16_split(s_eff, 3)
    sj = _bf16_split(s_eff + EPS_GUARD, 3)

    lhs = np.zeros((B, K_AUG, N), bf)
    rhs = np.zeros((B, K_AUG, N), bf)
    lhs[:, 0], lhs[:, 1], lhs[:, 2] = si
    rhs[:, 0:3] = 1.0
    for c in range(3):
        k = 3 + 4 * c
        m2ah = (-2.0 * ah[:, c].astype(np.float64)).astype(bf)
        m2al = (-2.0 * al[:, c].astype(np.float64)).astype(bf)
        lhs[:, k + 0], rhs[:, k + 0] = m2ah, ah[:, c]
        lhs[:, k + 1], rhs[:, k + 1] = m2ah, al[:, c]
        lhs[:, k + 2], rhs[:, k + 2] = m2al, ah[:, c]
        lhs[:, k + 3], rhs[:, k + 3] = m2al, al[:, c]
    lhs[:, 15:18] = 1.0
    rhs[:, 15], rhs[:, 16], rhs[:, 17] = sj

    k = np.arange(P)
    stepw = np.where(k[None, :] >= k[:, None], BIG, 0.0).astype(ml_dtypes.bfloat16)
    ident = np.eye(P).astype(ml_dtypes.bfloat16)

    in_maps = []
    for c in range(NCORES):
        in_maps.append({
            "lhs": np.ascontiguousarray(lhs[c * BPC:(c + 1) * BPC]),
            "rhs": np.ascontiguousarray(rhs[c * BPC:(c + 1) * BPC]),
            "stepw": stepw,
            "ident": ident,
        })
    return in_maps


def _run(coordinates, trace=False, **trace_kwargs):
    coords = np.asarray(coordinates, dtype=np.float32)
    assert coords.shape == (B, 3 * N), coords.shape
    nc = _build()
    in_maps = _prep_inputs(coords)
    res = run_bass_kernel_spmd(nc, in_maps, core_ids=list(range(NCORES)),
                               trace=trace, **trace_kwargs)
    total = 0.0
    for c in range(NCORES):
        total += np.sum(res.results[c]["stats"], dtype=np.float64)
    loss = np.float32(LOSS_WEIGHT * total / B)
    return loss, res


def kernel(coordinates):
    loss, _ = _run(coordinates)
    return np.asarray(loss, dtype=np.float32)


# revision 9
# speedup vs baseline: 1.1656x; 1.1656x over previous
"""Trainium2 Bass kernel for CollapsePreventionLoss.

reference:
    atoms = coordinates.reshape(B, N, 3)           # B=64, N=1024
    dist  = sqrt(pairwise_dist_sq + 1e-8)
    loss  = sum_{i<j} relu(2.9 - dist)^2 / B

Strategy (8 NeuronCores, data parallel over batch, 8 batches/core):
  dist_sq[i,j] = s_i + s_j - 2 a_i.a_j  as ONE K=18 bf16 matmul per tile:
  every product is exact in fp32 (bf16 hi/lo split of each coordinate,
  3-way bf16 split of the squared norms), so dist_sq is the exact pair
  distance of slightly-perturbed atoms, plus an EPS_GUARD that keeps it
  positive (sqrt(neg) = NaN on the ACT engine).

  Only upper-triangle block-rows are computed: row-block r (128 rows)
  covers columns [128r, 1024).  The 128x128 diagonal blocks are computed
  unmasked and accumulated separately; on the host the strict-upper part
  is recovered by symmetry: upper = (block_sum - diag_sum_estimate) / 2.

  Pipeline per PSUM tile: PE matmul chunks -> ACT sqrt (PSUM->SBUF, bf16)
  then per batch: DVE t = min(d-2.9, 0) (4x mode) and
  DVE scalar_tensor_tensor (d-2.9)*t with accum_out  ( == relu^2 sums).
  Host sums the [128, 16] per-core partials in fp64.
"""

import sys

for _p in ("/opt/trn_rl_repo",):
    if _p not in sys.path:
        sys.path.insert(0, _p)

import numpy as np

import concourse.bacc as bacc
import concourse.tile as tile
from concourse import mybir
from concourse.bass_utils import run_bass_kernel_spmd

B = 64
N = 1024
NCORES = 8
BPC = B // NCORES  # batches per core

MIN_DISTANCE = 2.9
LOSS_WEIGHT = 1.0
EPS_GUARD = 1e-4  # keeps dist_sq positive despite PSUM accumulation rounding
# (host emulation of the PE fp32 accumulation over this dataset bottoms out
# at dist_sq ~ -7.3e-6; 1e-4 gives >10x margin against sqrt(<0) = NaN)

# dist_sq is computed as an exact-in-fp32 bf16 hi/lo product expansion:
#   a ~= ah + al (bf16 pair), each product bf16 x bf16 is exact in fp32.
#   rows 0-2:   s_i  (3-way bf16 split)  x  1
#   rows 3-14:  4 products per coordinate: (-2ah,ah) (-2ah,al) (-2al,ah) (-2al,al)
#   rows 15-17: 1  x  (s_j + eps) (3-way bf16 split)
K_AUG = 18
P = 128
NRB = N // P  # row blocks per batch

# ---------------------------------------------------------------------------
# PSUM tile plan. Each tile is <= 1024 f32 (2 PSUM banks); chunks never cross
# a 512-col bank boundary.  Tile 0 holds the eight 128-wide diagonal blocks;
# the rest hold each row-block's off-diagonal columns [128(r+1), 1024).
# entries: (tile_width, [(row_block, col_start_local, width, col_start_global)])
TILES = [
    (1024, [(r, 128 * r, 128, 128 * r) for r in range(8)]),          # diagonals
    (896, [(0, 0, 512, 128), (0, 512, 384, 640)]),
    (768, [(1, 0, 512, 256), (1, 512, 256, 768)]),
    (768, [(2, 0, 512, 384), (2, 512, 128, 896), (6, 640, 128, 896)]),
    (768, [(3, 0, 512, 512), (5, 512, 256, 768)]),
    (384, [(4, 0, 384, 640)]),
]
D_W = sum(tw for tw, _ in TILES)  # 4608
D_OFF = np.cumsum([0] + [tw for tw, _ in TILES])[:-1]
DIAG_W = TILES[0][0]  # 1024: diagonal-block region at d[:, 0:DIAG_W]

_cache = {}


def _build():
    if "nc" in _cache:
        return _cache["nc"]
    f32 = mybir.dt.float32
    bf16 = mybir.dt.bfloat16

    nc = bacc.Bacc("TRN2", target_bir_lowering=False, debug=False,
                   num_devices=NCORES)
    lhs_d = nc.dram_tensor("lhs", [BPC, K_AUG, N], bf16, kind="ExternalInput").ap()
    rhs_d = nc.dram_tensor("rhs", [BPC, K_AUG, N], bf16, kind="ExternalInput").ap()
    stats_d = nc.dram_tensor("stats", [P, 2 * BPC], f32, kind="ExternalOutput").ap()

    with tile.TileContext(nc) as tc:
        with (
            tc.tile_pool(name="inp", bufs=4) as inp,
            tc.tile_pool(name="dpool", bufs=2) as dpool,
            tc.tile_pool(name="tpool", bufs=2) as tpool,
            tc.tile_pool(name="sqpool", bufs=2) as sqpool,
            tc.tile_pool(name="spool", bufs=1) as spool,
            tc.tile_pool(name="psum", bufs=4, space="PSUM") as psum,
        ):
            stats_sb = spool.tile([P, 2 * BPC], f32, tag="stats")

            for b in range(BPC):
                lhs_sb = inp.tile([K_AUG, N], bf16, tag="lhs_t")
                rhs_sb = inp.tile([K_AUG, N], bf16, tag="rhs_t")
                nc.sync.dma_start(out=lhs_sb, in_=lhs_d[b])
                nc.sync.dma_start(out=rhs_sb, in_=rhs_d[b])

                d_sb = dpool.tile([P, D_W], bf16, tag="d")
                for g, (tile_w, chunks) in enumerate(TILES):
                    pt = psum.tile([P, tile_w], f32, tag="pt")
                    for r, cs, w, jg in chunks:
                        nc.tensor.matmul(
                            pt[:, cs:cs + w],
                            lhs_sb[:, P * r:P * (r + 1)],
                            rhs_sb[:, jg:jg + w],
                            start=True,
                            stop=True,
                        )
                    nc.scalar.activation(
                        out=d_sb[:, int(D_OFF[g]):int(D_OFF[g]) + tile_w],
                        in_=pt[:, :],
                        func=mybir.ActivationFunctionType.Sqrt,
                        bias=0.0,  # EPS_GUARD is already inside dist_sq
                        scale=1.0,
                    )

                # t = min(d - 2.9, 0)   (== -relu(2.9 - d))
                t_sb = tpool.tile([P, D_W], bf16, tag="t")
                nc.vector.tensor_scalar(
                    out=t_sb,
                    in0=d_sb,
                    scalar1=float(MIN_DISTANCE),
                    scalar2=0.0,
                    op0=mybir.AluOpType.subtract,
                    op1=mybir.AluOpType.min,
                )
                # (d - 2.9) * t == t*t == relu(2.9-d)^2; accum_out sums it.
                # Diagonal blocks and off-diagonal blocks go to separate
                # stats columns (host halves the diagonal-block sum).
                sqd_sb = sqpool.tile([P, DIAG_W], f32, tag="sqd")
                nc.vector.scalar_tensor_tensor(
                    out=sqd_sb,
                    in0=d_sb[:, 0:DIAG_W],
                    scalar=float(MIN_DISTANCE),
                    in1=t_sb[:, 0:DIAG_W],
                    op0=mybir.AluOpType.subtract,
                    op1=mybir.AluOpType.mult,
                    accum_out=stats_sb[:, 2 * b:2 * b + 1],
                )
                sqo_sb = sqpool.tile([P, D_W - DIAG_W], mybir.dt.bfloat16,
                                     tag="sqo")
                nc.vector.scalar_tensor_tensor(
                    out=sqo_sb,
                    in0=d_sb[:, DIAG_W:D_W],
                    scalar=float(MIN_DISTANCE),
                    in1=t_sb[:, DIAG_W:D_W],
                    op0=mybir.AluOpType.subtract,
                    op1=mybir.AluOpType.mult,
                    accum_out=stats_sb[:, 2 * b + 1:2 * b + 2],
                )

            nc.sync.dma_start(out=stats_d, in_=stats_sb)

    nc.compile()
    _cache["nc"] = nc
    return nc


def _bf16_split(x, n):
    """Split fp64 array into n bf16 terms summing to ~x."""
    import ml_dtypes

    out = []
    rem = x.copy()
    for _ in range(n):
        h = rem.astype(ml_dtypes.bfloat16)
        out.append(h)
        rem = rem - h.astype(np.float64)
    return out


def _prep_inputs(coords):
    """Host-side: build augmented lhs/rhs per core (bf16 hi/lo expansion)."""
    import ml_dtypes

    bf = ml_dtypes.bfloat16
    atoms = coords.reshape(B, N, 3).astype(np.float64)
    at = atoms.transpose(0, 2, 1)  # [B, 3, N]
    ah = at.astype(bf)
    al = (at - ah.astype(np.float64)).astype(bf)
    a_eff = ah.astype(np.float64) + al.astype(np.float64)
    s_eff = (a_eff * a_eff).sum(axis=1)  # [B, N] exact squared norms of ã

    si = _bf16_split(s_eff, 3)
    sj = _bf16_split(s_eff + EPS_GUARD, 3)

    lhs = np.zeros((B, K_AUG, N), bf)
    rhs = np.zeros((B, K_AUG, N), bf)
    lhs[:, 0], lhs[:, 1], lhs[:, 2] = si
    rhs[:, 0:3] = 1.0
    for c in range(3):
        k = 3 + 4 * c
        m2ah = (-2.0 * ah[:, c].astype(np.float64)).astype(bf)
        m2al = (-2.0 * al[:, c].astype(np.float64)).astype(bf)
        lhs[:, k + 0], rhs[:, k + 0] = m2ah, ah[:, c]
        lhs[:, k + 1], rhs[:, k + 1] = m2ah, al[:, c]
        lhs[:, k + 2], rhs[:, k + 2] = m2al, ah[:, c]
        lhs[:, k + 3], rhs[:, k + 3] = m2al, al[:, c]
    lhs[:, 15:18] = 1.0
    rhs[:, 15], rhs[:, 16], rhs[:, 17] = sj

    in_maps = []
    for c in range(NCORES):
        in_maps.append({
            "lhs": np.ascontiguousarray(lhs[c * BPC:(c + 1) * BPC]),
            "rhs": np.ascontiguousarray(rhs[c * BPC:(c + 1) * BPC]),
        })
    return in_maps


def _diag_estimate():
    """Expected kernel-computed sum over the 1024 true-diagonal elements of
    one batch's diagonal blocks: d_ii = sqrt(EPS_GUARD) -> bf16, then
    (d - 2.9) * bf16(d - 2.9)."""
    import ml_dtypes

    bf = ml_dtypes.bfloat16
    d = float(np.float64(np.sqrt(EPS_GUARD)).astype(bf))
    t = float(np.float64(d - MIN_DISTANCE).astype(bf))
    return N * (d - MIN_DISTANCE) * t


def _run(coordinates, trace=False, **trace_kwargs):
    coords = np.asarray(coordinates, dtype=np.float32)
    assert coords.shape == (B, 3 * N), coords.shape
    nc = _build()
    in_maps = _prep_inputs(coords)
    res = run_bass_kernel_spmd(nc, in_maps, core_ids=list(range(NCORES)),
                               trace=trace, **trace_kwargs)
    diag_est = _diag_estimate()
    total = 0.0
    for c in range(NCORES):
        st = res.results[c]["stats"].astype(np.float64)
        for b in range(BPC):
            s_diag = st[:, 2 * b].sum()
            s_off = st[:, 2 * b + 1].sum()
            total += s_off + 0.5 * (s_diag - diag_est)
    loss = np.float32(LOSS_WEIGHT * total / B)
    return loss, res


def kernel(coordinates):
    loss, _ = _run(coordinates)
    return np.asarray(loss, dtype=np.float32)
